# revision 17
# baseline (speedup 1.0000x reference)
"""Trainium2 Bass kernel: CrossAttnBlock (16x4096x512 query, 16x77x768 cond).

Sharding: pure data-parallel over batch -- 2 batches per core on 8 cores,
no collectives.  Host-side work is layout-only (transposes / slicing /
bf16 casts).

v2: all matmuls in bf16 (fp32_mode=HIGH ran the PE at <1/2 rate and
blocked fast-weight-load), bf16 activations end to end, bf16 output
upcast on the host.  The softmax reciprocal moved off the scalar engine
(Ln/Exp alternation forced two 1.3us activation-table loads per chunk)
onto DVE's approximate-reciprocal path, in a [64,64] layout so the
free-dim cost is 64 not 512.  PSUM evacuations are split between the
scalar and vector engines; the second normalize multiply and the small
reciprocal cast run on gpsimd (which cannot touch PSUM, so it only gets
SBUF-to-SBUF work).

On-device dataflow per core (activations kept transposed: feature dim on
SBUF partitions, tokens on the free dim):
    qT = wqT-stationary matmuls over xT chunks  [512f x 512t] per chunk
    kT = wkT-stationary matmuls over condT      [512f x 154s]
    v  = condT-stationary matmuls               [77s x 512d] -> v_aug [77, 8*65]
    scoresT_h = kT_h stationary @ qT_h          [77s x 512t]  (row-packed pairs)
    e_h = exp(scoresT_h / 8)                    (no max subtraction; scores ~ +-2)
    avT_h = v_aug_h @ e_h                       [65, 512t] (row 64 = denominator)
    normalize: DVE approx-recip + bf16 DRAM-bounce broadcast + DVE/gpsimd mul
    yT = woT-stationary matmuls over normalized avT

The emission order is software-pipelined across chunks so each engine's
in-order stream has its cross-engine dependencies already satisfied:
    iter i:  dma(i+1) | Qproj(i) | scores+exp(i-1) | Oproj(i-2) | av+norm(i-1)
"""

import os
import numpy as np

MODEL_DIM = 512
COND_DIM = 768
HEAD_DIM = 64
N_HEADS = 8
B = 16
T = 4096
LK = 77
N_CORES = 8
NB = B // N_CORES          # batches per core
CHUNK = 512                # tokens per chunk
NCHUNK = T // CHUNK
NCTOT = NB * NCHUNK        # total chunks per core
KD = MODEL_DIM // 128      # 4 partition tiles of model dim
CDT = COND_DIM // 128      # 6 partition tiles of cond dim
SCALE = HEAD_DIM ** -0.5

_PROG = None               # cached compiled Bass program
LAST_RESULTS = None        # BassKernelResults of last run (for profiling)


def _build_program():
    import concourse.bass as bass  # noqa: F401
    import concourse.tile as tile
    from concourse import bacc, mybir
    from contextlib import ExitStack

    f32 = mybir.dt.float32
    bf16 = mybir.dt.bfloat16
    Exp = mybir.ActivationFunctionType.Exp

    nc = bacc.Bacc(
        "TRN2", target_bir_lowering=False, debug=False, num_devices=N_CORES
    )

    xt = nc.dram_tensor("xt", [NB, MODEL_DIM, T], bf16, kind="ExternalInput").ap()
    condt = nc.dram_tensor(
        "condt", [COND_DIM, NB * LK], bf16, kind="ExternalInput"
    ).ap()
    wqt = nc.dram_tensor("wqt", [MODEL_DIM, MODEL_DIM], bf16, kind="ExternalInput").ap()
    wkt = nc.dram_tensor("wkt", [COND_DIM, MODEL_DIM], bf16, kind="ExternalInput").ap()
    wvt = nc.dram_tensor("wvt", [COND_DIM, MODEL_DIM], bf16, kind="ExternalInput").ap()
    wot = nc.dram_tensor("wot", [MODEL_DIM, MODEL_DIM], bf16, kind="ExternalInput").ap()
    yt = nc.dram_tensor("yt", [NB, MODEL_DIM, T], bf16, kind="ExternalOutput").ap()

    with tile.TileContext(nc) as tc, ExitStack() as ctx:
        wp = ctx.enter_context(tc.tile_pool(name="wp", bufs=1))
        bp = ctx.enter_context(tc.tile_pool(name="bp", bufs=1))   # per-batch stuff
        xp = ctx.enter_context(tc.tile_pool(name="xp", bufs=2))   # x chunks
        qp = ctx.enter_context(tc.tile_pool(name="qp", bufs=2))   # qT chunks
        epool = ctx.enter_context(tc.tile_pool(name="epool", bufs=2))
        avp = ctx.enter_context(tc.tile_pool(name="avp", bufs=2))  # evac'd attnV
        rp = ctx.enter_context(tc.tile_pool(name="rp", bufs=2))    # denom [64,64]
        rtp = ctx.enter_context(tc.tile_pool(name="rtp", bufs=2))  # recip f32/bf16
        bcp = ctx.enter_context(tc.tile_pool(name="bcp", bufs=2))  # broadcasts
        drp = ctx.enter_context(tc.tile_pool(name="drp", bufs=2, space="DRAM"))
        npool = ctx.enter_context(tc.tile_pool(name="npool", bufs=2))
        yp = ctx.enter_context(tc.tile_pool(name="yp", bufs=2))
        # PSUM (8 banks): one deep [128,512] ring shared by Q-proj, attn@V
        # and O-proj tiles (4 banks), and one deep [77,512] scores ring (4
        # banks).  bufs=4 gives every PSUM release >=8 matmuls of slack, so
        # the PE never waits on an evacuation that is still queued behind
        # slower engine work.
        pqy = ctx.enter_context(tc.tile_pool(name="pqy", bufs=4, space="PSUM"))
        ps = ctx.enter_context(tc.tile_pool(name="ps", bufs=4, space="PSUM"))

        # ---- load weights ----
        def load_rows(dram_ap, n_tiles, free, tagbase):
            tiles = []
            for k in range(n_tiles):
                t_ = wp.tile([128, free], bf16, tag=f"{tagbase}{k}", name=f"{tagbase}{k}")
                nc.sync.dma_start(out=t_, in_=dram_ap[k * 128:(k + 1) * 128, :])
                tiles.append(t_)
            return tiles

        wq_sb = load_rows(wqt, KD, MODEL_DIM, "wq")
        wo_sb = load_rows(wot, KD, MODEL_DIM, "wo")

        # ---- setup-only weights live in a scoped pool, released after ----
        sp_setup = tc.alloc_tile_pool(name="sp_setup", bufs=1)
        def load_rows_setup(dram_ap, n_tiles, free, tagbase):
            tiles = []
            for k in range(n_tiles):
                t_ = sp_setup.tile([128, free], bf16, tag=f"{tagbase}{k}",
                                   name=f"{tagbase}{k}")
                nc.sync.dma_start(out=t_, in_=dram_ap[k * 128:(k + 1) * 128, :])
                tiles.append(t_)
            return tiles

        wk_sb = load_rows_setup(wkt, CDT, MODEL_DIM, "wk")
        wv_sb = load_rows_setup(wvt, CDT, MODEL_DIM, "wv")
        cond_sb = load_rows_setup(condt, CDT, NB * LK, "cond")

        # ---- K projection (both batches at once): kT [512, NB*77] ----
        kt_sb = []
        for f in range(KD):
            psk = pqy.tile([128, NB * LK], f32, tag="qy", name=f"psk{f}")
            for c in range(CDT):
                nc.tensor.matmul(
                    psk,
                    lhsT=wk_sb[c][:, f * 128:(f + 1) * 128],
                    rhs=cond_sb[c],
                    start=(c == 0),
                    stop=(c == CDT - 1),
                )
            ktf = bp.tile([128, NB * LK], bf16, tag=f"kt{f}", name=f"kt{f}")
            nc.scalar.copy(ktf, psk)
            kt_sb.append(ktf)

        # ---- V projection per batch -> v_aug [77, 8*65] (65th col = ones) ----
        v_aug = []
        for b in range(NB):
            psv = ps.tile([LK, MODEL_DIM], f32, tag="s", name=f"psv{b}")
            for c in range(CDT):
                nc.tensor.matmul(
                    psv,
                    lhsT=cond_sb[c][:, b * LK:(b + 1) * LK],
                    rhs=wv_sb[c],
                    start=(c == 0),
                    stop=(c == CDT - 1),
                )
            va = bp.tile([LK, N_HEADS * (HEAD_DIM + 1)], bf16, tag=f"va{b}",
                         name=f"va{b}")
            for h in range(N_HEADS):
                nc.scalar.copy(
                    va[:, h * 65:h * 65 + 64], psv[:, h * 64:(h + 1) * 64]
                )
            ones_view = va.rearrange("p (h c) -> p h c", c=65)[:, :, 64]
            nc.vector.memset(ones_view, 1.0)
            v_aug.append(va)
        sp_setup.release()

        # ---- software-pipelined main loop ----
        # chunk g: batch b = g // NCHUNK, token offset t0 = (g % NCHUNK)*CHUNK
        xt_sb = {}    # g -> single [128, 4*CHUNK] tile
        q_sb = {}     # g -> list of 4 qT tiles
        exp_sb = {}   # g -> list of 8 exp tiles
        norm_sb = {}  # g -> normalized avT tile [128, 4*CHUNK]

        def dma_load(g):
            b, t0 = g // NCHUNK, (g % NCHUNK) * CHUNK
            xk = xp.tile([128, KD * CHUNK], bf16, tag="xt", name=f"x{g}")
            base = xt[b, 0:1, t0:t0 + 1]
            src = bass.AP(base.tensor, base.offset,
                          [[T, 128], [128 * T, KD], [1, CHUNK]])
            nc.sync.dma_start(out=xk, in_=src)
            xt_sb[g] = xk

        avsb_live = {}   # g -> avsb pair (kept until the deferred norm muls)
        bc_live = {}     # g -> bc tile

        def scores_head(g, h):
            b = g // NCHUNK
            p, half = h // 2, h % 2
            lo, hi = 64 * half, 64 * (half + 1)
            pss = ps.tile([LK, CHUNK], f32, tag="s", name=f"pss{g}_{h}")
            nc.tensor.matmul(
                pss,
                lhsT=kt_sb[p][lo:hi, b * LK:(b + 1) * LK],
                rhs=q_sb[g][p][lo:hi, :],
                start=True,
                stop=True,
            )
            e = epool.tile([LK, CHUNK], bf16, tag=f"e{h}", name=f"e{g}_{h}")
            nc.scalar.activation(e, pss, Exp, scale=SCALE)
            exp_sb.setdefault(g, []).append(e)
            if h == N_HEADS - 1:
                del q_sb[g]

        def av_head(g, h):
            b = g // NCHUNK
            avsb = avsb_live[g]
            pavt = pqy.tile([128, CHUNK], f32, tag="qy", name=f"pav{g}_{h}")
            nc.tensor.matmul(
                pavt[0:HEAD_DIM + 1, :],
                lhsT=v_aug[b][:, h * 65:(h + 1) * 65],
                rhs=exp_sb[g][h],
                start=True,
                stop=True,
            )
            dst = avsb[h % 2][:, (h // 2) * CHUNK:(h // 2 + 1) * CHUNK]
            if h in (0, 2, 4):
                nc.scalar.copy(dst, pavt[0:HEAD_DIM + 1, :])
            else:
                nc.vector.tensor_copy(dst, pavt[0:HEAD_DIM + 1, :])

        def qproj_f(g, f):
            psq = pqy.tile([128, CHUNK], f32, tag="qy", name=f"psq{g}_{f}")
            for k in range(KD):
                nc.tensor.matmul(
                    psq,
                    lhsT=wq_sb[k][:, f * 128:(f + 1) * 128],
                    rhs=xt_sb[g][:, k * CHUNK:(k + 1) * CHUNK],
                    start=(k == 0),
                    stop=(k == KD - 1),
                )
            qf = qp.tile([128, CHUNK], bf16, tag=f"q{f}", name=f"q{g}_{f}")
            nc.vector.tensor_copy(qf, psq)
            q_sb.setdefault(g, []).append(qf)
            if f == KD - 1:
                del xt_sb[g]

        def oproj_f(g, f, ysb):
            psy = pqy.tile([128, CHUNK], f32, tag="qy", name=f"psy{g}_{f}")
            for j in range(KD):
                nc.tensor.matmul(
                    psy,
                    lhsT=wo_sb[j][:, f * 128:(f + 1) * 128],
                    rhs=norm_sb[g][:, j * CHUNK:(j + 1) * CHUNK],
                    start=(j == 0),
                    stop=(j == KD - 1),
                )
            dst = ysb[:, f * CHUNK:(f + 1) * CHUNK]
            if f % 2 == 0:
                nc.scalar.copy(dst, psy)
            else:
                nc.vector.tensor_copy(dst, psy)

        def recip_chain(g):
            # denominators: avsb[i] row 64 is [1, 2048] = 4 heads x 512
            # tokens; reshape each into 32 partitions x 64 cols.
            avsb = avsb_live[g]
            dt_ = rp.tile([64, 64], bf16, tag="dt", name=f"dt{g}")
            for i in range(2):
                nc.sync.dma_start(out=dt_[32 * i:32 * i + 32, :],
                                  in_=avsb[i][HEAD_DIM:HEAD_DIM + 1, :])
            dtf = rtp.tile([64, 64], f32, tag="dtf", name=f"dtf{g}")
            nc.gpsimd.tensor_copy(dtf, dt_)
            rtf = rtp.tile([64, 64], f32, tag="rtf", name=f"rtf{g}")
            nc.vector.reciprocal_approx_fast(rtf, dtf)
            rt = rtp.tile([64, 64], bf16, tag="rt", name=f"rt{g}")
            nc.gpsimd.tensor_copy(rt, rtf)
            # bounce through DRAM, then one replicating read:
            # scr layout: group i at [i, :] = (head pair slot, token) flat
            scr = drp.tile([2, KD * CHUNK], bf16, tag="scr", name=f"scr{g}")
            for i in range(2):
                nc.sync.dma_start(out=scr[i:i + 1, :],
                                  in_=rt[32 * i:32 * i + 32, :])
            # bc: [64, 2*2048] -- group i's replicated reciprocals at
            # cols [i*2048, (i+1)*2048), so both multiplies read
            # base-partition-0 inputs (a hardware requirement).
            bc = bcp.tile([HEAD_DIM, 2 * KD * CHUNK], bf16, tag="bc", name=f"bc{g}")
            base = scr[0:1, 0:1]
            rep = bass.AP(base.tensor, base.offset,
                          [[0, HEAD_DIM], [KD * CHUNK, 2], [1, KD * CHUNK]])
            nc.sync.dma_start(out=bc, in_=rep)
            bc_live[g] = bc
            del exp_sb[g]

        def norm_muls(g):
            # Emitted at the tail of the iteration: bc(g) is ready (or about
            # to be), and the consumer (O-proj of chunk g) is a full
            # iteration away, so these never head-of-line block anything.
            avsb = avsb_live.pop(g)
            bc = bc_live.pop(g)
            norm = npool.tile([128, KD * CHUNK], bf16, tag="n", name=f"n{g}")
            nc.gpsimd.tensor_mul(
                norm[0:HEAD_DIM, 0:2 * CHUNK], avsb[0][0:HEAD_DIM, 0:2 * CHUNK],
                bc[:, 0:2 * CHUNK]
            )
            nc.vector.tensor_mul(
                norm[0:HEAD_DIM, 2 * CHUNK:KD * CHUNK],
                avsb[0][0:HEAD_DIM, 2 * CHUNK:KD * CHUNK],
                bc[:, 2 * CHUNK:KD * CHUNK]
            )
            nc.vector.tensor_mul(
                norm[HEAD_DIM:128, :], avsb[1][0:HEAD_DIM, :],
                bc[:, KD * CHUNK:2 * KD * CHUNK]
            )
            norm_sb[g] = norm

        def iteration(i):
            gq = i if i < NCTOT else None            # Q-proj chunk
            gs = i - 1 if 1 <= i < NCTOT + 1 else None   # scores/exp chunk
            ga = i - 2 if 2 <= i < NCTOT + 2 else None   # attn@V chunk
            go = i - 3 if 3 <= i else None           # O-proj chunk
            if i + 1 < NCTOT:
                dma_load(i + 1)
            if ga is not None:
                avsb_live[ga] = [
                    avp.tile([HEAD_DIM + 1, KD * CHUNK], bf16, tag=f"av{j}",
                             name=f"avsb{ga}_{j}")
                    for j in range(2)
                ]
            ysb = None
            if go is not None:
                ysb = yp.tile([128, KD * CHUNK], bf16, tag="y", name=f"y{go}")
            # strict alternation [attn@V head | proj burst | scores head]:
            # consecutive uses of any PSUM ring slot are ~4 units apart, so
            # evacuations queued on ACT/DVE always beat the slot reuse.
            for u in range(N_HEADS):
                if ga is not None:
                    av_head(ga, u)
                if u < KD:
                    if gq is not None:
                        qproj_f(gq, u)
                else:
                    if go is not None:
                        oproj_f(go, u - KD, ysb)
                if gs is not None:
                    scores_head(gs, u)
            if go is not None:
                b, t0 = go // NCHUNK, (go % NCHUNK) * CHUNK
                base = yt[b, 0:1, t0:t0 + 1]
                dst_ap = bass.AP(base.tensor, base.offset,
                                 [[T, 128], [128 * T, KD], [1, CHUNK]])
                nc.sync.dma_start(out=dst_ap, in_=ysb)
                del norm_sb[go]
            if ga is not None:
                recip_chain(ga)
                norm_muls(ga)

        dma_load(0)
        for i in range(NCTOT + 3):
            iteration(i)

    nc.compile()
    return nc


def _get_program():
    global _PROG
    if _PROG is None:
        _PROG = _build_program()
    return _PROG


def _shard_inputs(x, cond, w_q, w_k, w_v, w_o):
    """Host-side layout: transpose + bf16 cast + shard."""
    import ml_dtypes

    bf = ml_dtypes.bfloat16
    wqt = np.ascontiguousarray(w_q.T).astype(bf)
    wkt = np.ascontiguousarray(w_k.T).astype(bf)
    wvt = np.ascontiguousarray(w_v.T).astype(bf)
    wot = np.ascontiguousarray(w_o.T).astype(bf)

    xT = np.ascontiguousarray(x.transpose(0, 2, 1)).astype(bf)       # [B, D, T]
    condT = np.ascontiguousarray(cond.transpose(0, 2, 1)).astype(bf)  # [B, CD, LK]

    in_maps = []
    for c in range(N_CORES):
        b0 = c * NB
        ct = np.ascontiguousarray(
            condT[b0:b0 + NB].transpose(1, 0, 2).reshape(COND_DIM, NB * LK)
        )
        in_maps.append(
            {
                "xt": np.ascontiguousarray(xT[b0:b0 + NB]),
                "condt": ct,
                "wqt": wqt,
                "wkt": wkt,
                "wvt": wvt,
                "wot": wot,
            }
        )
    return in_maps


def kernel(x, cond, w_q, w_k, w_v, w_o):
    global LAST_RESULTS
    from concourse.bass_utils import run_bass_kernel_spmd

    nc = _get_program()
    in_maps = _shard_inputs(x, cond, w_q, w_k, w_v, w_o)
    trace = bool(os.environ.get("BASS_TRACE"))
    res = run_bass_kernel_spmd(
        nc, in_maps, list(range(N_CORES)), trace=trace
    )
    LAST_RESULTS = res

    out = np.empty((B, T, MODEL_DIM), dtype=np.float32)
    for c in range(N_CORES):
        ytc = np.asarray(res.results[c]["yt"]).astype(np.float32)  # [NB, D, T]
        out[c * NB:(c + 1) * NB] = ytc.transpose(0, 2, 1)
    return out
